# revision 7
# baseline (speedup 1.0000x reference)
"""KDE2D Trainium2 Bass kernel — Fourier (trig-moment) factorization.

Reference (per (b,t), B=16, T=64, N=512, grid 128x128, bandwidth h):
  standardize points (mean/std ddof=1 over N), then
  density[g,h] = 1/(2 pi h^2) * sum_n exp(-(xg-x_n)^2/2h^2) * exp(-(yg-y_n)^2/2h^2)
              = sum_n phi(xg - x_n) * phi(yg - y_n),   phi = 1D-normalized Gaussian.

Kernel idea: periodize phi with period L and truncate its Fourier series at K
harmonics.  With theta = 2*pi*x/L and the D = 2K+1 feature vector
v(x) = [1, cos(j*theta), sin(j*theta)]_{j=1..K}:
  phi(g - x) ~= sum_d U[d, g] * v_d(x)       (U solved host-side by lstsq)
  density    = Ux^T M Uy,   M[d, e] = sum_n v_d(x_n) v_e(y_n).
Per (b,t) the device work collapses to tiny matmuls:
  M2 = Vy^T Vx   (PE, contract n in 4 chunks of 128)
  Z  = M2^T-contract: lhsT=M2[e,d], rhs=U  -> Z[d, h]
  D  = lhsT=U[d, g] (const stationary), rhs=Z -> density[g, h]
V tiles [n, D] are built once per core for all 128 bt via half-angle Sin
seeds (ACT) + Chebyshev recurrences (DVE/Pool) in fp16 — no per-point exp.
Output is written f16 and upcast on host (halves output DMA traffic).

Validated in numpy: K=8, L=11 gives rel-Frobenius ~3e-3 (gate 2e-2).
"""

import math

import numpy as np

import concourse.bass as bass
import concourse.bacc as bacc
import concourse.mybir as mybir
from concourse import tile
from concourse.bass_utils import run_bass_kernel_spmd

B, T, N, GRID = 16, 64, 512, 128
NCORES = 8
BT_PER_CORE = (B * T) // NCORES  # 128
NCHUNK = N // 128  # 4

KHARM = 8            # Fourier harmonics
LPER = 11.0          # periodization length
DDIM = 2 * KHARM + 1  # 17 feature dims

F32 = mybir.dt.float32
F16 = mybir.dt.float16

_CACHE = {}


def _build(bw: float):
    AT = mybir.ActivationFunctionType
    OP = mybir.AluOpType

    nc = bacc.Bacc("TRN2", target_bir_lowering=False)
    a_ext = nc.declare_dram_parameter("a", [BT_PER_CORE, N, 2], F32, isOutput=False)
    idt_ext = nc.declare_dram_parameter("idt", [128, 128], F32, isOutput=False)
    u_ext = nc.declare_dram_parameter("umat", [DDIM, GRID], F16, isOutput=False)
    out_ext = nc.declare_dram_parameter(
        "out", [BT_PER_CORE, GRID, GRID], F16, isOutput=True
    )

    two_pi_over_l = 2.0 * math.pi / LPER

    with tile.TileContext(nc) as tc:
        with (
            tc.tile_pool(name="const", bufs=1) as cpool,
            tc.tile_pool(name="stats", bufs=1) as spool,
            tc.tile_pool(name="vpool", bufs=1) as vpool,
            tc.tile_pool(name="work", bufs=2) as wpool,
            tc.tile_pool(name="ptr", bufs=1, space="PSUM") as trpool,
            tc.tile_pool(name="pm", bufs=2, space="PSUM") as mpool,
            tc.tile_pool(name="pz", bufs=2, space="PSUM") as zpool,
            tc.tile_pool(name="pd", bufs=2, space="PSUM") as dpool,
            tc.tile_pool(name="msb", bufs=2) as msbpool,
            tc.tile_pool(name="zsb", bufs=2) as zsbpool,
            tc.tile_pool(name="dsb", bufs=2) as dsbpool,
        ):
            idt_sb = cpool.tile([128, 128], F32, tag="idt")
            u_sb = cpool.tile([DDIM, GRID], F16, tag="umat")
            halfpi = cpool.tile([128, 1], F32, tag="halfpi")
            nc.sync.dma_start(idt_sb[:], idt_ext[:])
            nc.sync.dma_start(u_sb[:], u_ext[:])
            nc.vector.memset(halfpi[:], math.pi / 2.0)

            a_all = spool.tile([128, N, 2], F32, tag="a")
            nc.sync.dma_start(a_all[:], a_ext[:])

            # ---- per-bt stats; theta = (x - mean) * invsd * (2 pi / L) ----
            # layout [bt(128 part), n(512 free)]
            theta = {}
            for ch, ci in (("x", 0), ("y", 1)):
                src = a_all[:, :, ci]
                s1 = spool.tile([128, 1], F32, tag=f"s1{ch}")
                s2 = spool.tile([128, 1], F32, tag=f"s2{ch}")
                sq = wpool.tile([128, N], F32, tag=f"sq{ch}")
                nc.vector.tensor_reduce(s1[:], src, mybir.AxisListType.X, OP.add)
                nc.vector.tensor_tensor(sq[:], src, src, OP.mult)
                nc.vector.tensor_reduce(s2[:], sq[:], mybir.AxisListType.X, OP.add)
                mean = spool.tile([128, 1], F32, tag=f"mean{ch}")
                nc.vector.tensor_scalar_mul(mean[:], s1[:], 1.0 / N)
                m2 = spool.tile([128, 1], F32, tag=f"m2{ch}")
                nc.vector.tensor_tensor(m2[:], mean[:], mean[:], OP.mult)
                var = spool.tile([128, 1], F32, tag=f"var{ch}")
                nc.vector.scalar_tensor_tensor(
                    var[:], m2[:], -float(N), s2[:], OP.mult, OP.add
                )
                nc.vector.tensor_scalar_mul(var[:], var[:], 1.0 / (N - 1))
                sd = spool.tile([128, 1], F32, tag=f"sd{ch}")
                nc.scalar.activation(sd[:], var[:], AT.Sqrt)
                invsd = spool.tile([128, 1], F32, tag=f"invsd{ch}")
                nc.vector.reciprocal(invsd[:], sd[:])
                alph = spool.tile([128, 1], F32, tag=f"alph{ch}")
                nc.vector.tensor_scalar_mul(alph[:], invsd[:], two_pi_over_l)
                th = spool.tile([128, N], F32, tag=f"th{ch}")
                # (x - mean) * alpha in one two-stage tensor_scalar
                nc.vector.tensor_scalar(
                    th[:], src, mean[:, 0:1], alph[:, 0:1], OP.subtract, OP.mult
                )
                theta[ch] = th

            # ---- transpose theta to [n(128 part), (chunk, bt)] ----
            thT = {}
            for ch in ("x", "y"):
                pt = trpool.tile([128, NCHUNK, 128], F32, tag=f"pt{ch}")
                for cc in range(NCHUNK):
                    nc.tensor.transpose(
                        pt[:, cc, :], theta[ch][:, cc * 128 : (cc + 1) * 128], idt_sb[:]
                    )
                tt = vpool.tile([128, NCHUNK, 128], F32, tag=f"thT{ch}")
                nc.vector.tensor_copy(tt[:], pt[:])
                thT[ch] = tt

            # ---- V tiles [n part, d, chunk, bt] f16 via seeds + Chebyshev ----
            # v_0 = 1, v_j = cos(j th), v_{K+j} = sin(j th)
            V = {}
            for ch in ("x", "y"):
                v = vpool.tile([128, DDIM, NCHUNK, 128], F16, tag=f"V{ch}")
                nc.vector.memset(v[:, 0], 1.0)
                hs = wpool.tile([128, NCHUNK, 128], F16, tag=f"hs{ch}")
                hc = wpool.tile([128, NCHUNK, 128], F16, tag=f"hc{ch}")
                # half-angle seeds keep Sin args within [-pi, pi]
                nc.scalar.activation(hs[:], thT[ch][:], AT.Sin, scale=0.5)
                nc.scalar.activation(
                    hc[:], thT[ch][:], AT.Sin, bias=halfpi[:, 0:1], scale=0.5
                )
                # sin th = 2 hs hc ; cos th = 1 - 2 hs^2
                prod = wpool.tile([128, NCHUNK, 128], F16, tag=f"pr{ch}")
                nc.vector.tensor_tensor(prod[:], hs[:], hc[:], OP.mult)
                nc.vector.tensor_scalar(
                    v[:, KHARM + 1], prod[:], 2.0, None, OP.mult
                )
                nc.vector.tensor_tensor(prod[:], hs[:], hs[:], OP.mult)
                nc.vector.tensor_scalar(
                    v[:, 1], prod[:], -2.0, 1.0, OP.mult, OP.add
                )
                t2 = wpool.tile([128, NCHUNK, 128], F16, tag=f"t2{ch}")
                nc.vector.tensor_scalar(t2[:], v[:, 1], 2.0, None, OP.mult)
                V[ch] = (v, t2)

            vx, _ = V["x"]
            vy, _ = V["y"]

            # Chebyshev: v_{j} = t2 * v_{j-1} - v_{j-2} for both cos and sin
            # chains, emitted per bt-half so the first half of the bt loop can
            # overlap with the second half of the V build.
            def emit_recurrence(half):
                sl = slice(half * 64, half * 64 + 64)
                ops = []
                for j in range(2, KHARM + 1):
                    for ch in ("x", "y"):
                        v, t2 = V[ch]
                        for base in (0, KHARM):
                            jm1 = base + j - 1
                            jm2 = base + j - 2
                            dst = base + j
                            sin_start = base and j == 2  # sin_0 = 0
                            ops.append((ch, t2, v, jm1, jm2, dst, sin_start))
                insts = []
                for ch, t2, v, jm1, jm2, dst, sin_start in ops:
                    u = wpool.tile([128, NCHUNK, 64], F16, tag=f"u{ch}{dst}{half}")
                    insts.append(
                        ("tt", u[:], t2[:, :, sl], v[:, jm1, :, sl], OP.mult)
                    )
                    if sin_start:
                        insts.append(("cp", v[:, dst, :, sl], u[:]))
                    else:
                        insts.append(
                            ("tt", v[:, dst, :, sl], u[:], v[:, jm2, :, sl], OP.subtract)
                        )
                return insts

            def run_inst(inst, eng):
                if inst[0] == "tt":
                    eng.tensor_tensor(inst[1], inst[2], inst[3], inst[4])
                else:
                    eng.tensor_copy(inst[1], inst[2])

            for inst in emit_recurrence(0):
                run_inst(inst, nc.vector)
            pending = emit_recurrence(1)

            # ---- main loop: groups of 4 bt; DMA out in 8-bt batches ----
            NG = BT_PER_CORE // 4
            dsb = None
            for gi in range(NG):
                # interleave second-half V ops with the first half of the loop
                if gi < 16:
                    for _ in range(7):
                        if pending:
                            run_inst(pending.pop(0), nc.vector)
                mps = mpool.tile([DDIM, 4, DDIM], F32, tag="mps")
                for i in range(4):
                    bt = gi * 4 + i
                    for cc in range(NCHUNK):
                        # M2[e,d] = sum_n Vy[n,e] Vx[n,d]
                        nc.tensor.matmul(
                            mps[:, i, :],
                            vy[:, :, cc, bt],
                            vx[:, :, cc, bt],
                            start=(cc == 0),
                            stop=(cc == NCHUNK - 1),
                        )
                msb = msbpool.tile([DDIM, 4, DDIM], F16, tag="msb")
                nc.vector.tensor_copy(msb[:], mps[:])

                zps = zpool.tile([DDIM, 4, GRID], F32, tag="zps")
                for i in range(4):
                    # Z[d, h] = sum_e M2[e, d] U[e, h]
                    nc.tensor.matmul(
                        zps[:, i, :], msb[:, i, :], u_sb[:], start=True, stop=True
                    )
                zsb = zsbpool.tile([DDIM, 4, GRID], F16, tag="zsb")
                if gi % 2 == 0:
                    nc.scalar.activation(zsb[:], zps[:], AT.Copy)
                else:
                    nc.vector.tensor_copy(zsb[:], zps[:])

                dps = dpool.tile([128, 4, GRID], F32, tag="dps")
                # density[g, h] = sum_d U[d, g] Z[d, h] — one matmul per group
                nc.tensor.matmul(dps[:], u_sb[:], zsb[:], start=True, stop=True)
                if gi % 2 == 0:
                    dsb = dsbpool.tile([128, 8, GRID], F16, tag="dsb")
                half = dsb[:, (gi % 2) * 4 : (gi % 2) * 4 + 4, :]
                if gi % 2 == 0:
                    nc.vector.tensor_copy(half, dps[:])
                else:
                    nc.scalar.activation(half, dps[:], AT.Copy)
                if gi % 2 == 1:
                    dst = out_ext[(gi - 1) * 4 : (gi + 1) * 4].transpose([1, 0, 2])
                    nc.sync.dma_start(dst, dsb[:])
            while pending:
                run_inst(pending.pop(0), nc.vector)

    if not nc.is_finalized():
        nc.finalize()
    return nc


def _consts(bw: float):
    h = float(bw)
    g = np.linspace(-5.0, 5.0, GRID)
    xs = np.linspace(-5.1, 5.1, 4001)
    th = 2.0 * np.pi * xs / LPER
    cols = (
        [np.ones_like(th)]
        + [np.cos(j * th) for j in range(1, KHARM + 1)]
        + [np.sin(j * th) for j in range(1, KHARM + 1)]
    )
    Phi = np.stack(cols, axis=-1)  # [S, D]
    Tgt = np.exp(-((g[None, :] - xs[:, None]) ** 2) / (2.0 * h * h)) / (
        np.sqrt(2.0 * np.pi) * h
    )
    AtA = Phi.T @ Phi + 1e-7 * len(xs) * np.eye(DDIM)
    U = np.linalg.solve(AtA, Phi.T @ Tgt)  # [D, G]
    idt = np.eye(128, dtype=np.float32)
    return idt, U.astype(np.float16)


def kernel(A: np.ndarray, bandwidth: np.ndarray) -> np.ndarray:
    A = np.asarray(A, dtype=np.float32)
    bw = float(np.asarray(bandwidth))
    key = round(bw, 9)
    if key not in _CACHE:
        _CACHE[key] = _build(bw)
    nc = _CACHE[key]

    idt, umat = _consts(bw)
    a_flat = A.reshape(B * T, N, 2)
    in_maps = []
    for i in range(NCORES):
        in_maps.append(
            {
                "a": np.ascontiguousarray(
                    a_flat[i * BT_PER_CORE : (i + 1) * BT_PER_CORE]
                ),
                "idt": idt,
                "umat": umat,
            }
        )
    res = run_bass_kernel_spmd(nc, in_maps, core_ids=list(range(NCORES)))
    outs = [res.results[i]["out"] for i in range(NCORES)]
    return (
        np.concatenate(outs, axis=0).astype(np.float32).reshape(B, T, GRID, GRID)
    )


if __name__ == "__main__":
    Arand = np.random.randn(B, T, N, 2).astype(np.float32)
    out = kernel(Arand, np.float32(0.5))
    print(out.shape, out.dtype, float(out.max()))


# revision 17
# speedup vs baseline: 1.0539x; 1.0539x over previous
"""KDE2D Trainium2 Bass kernel — Fourier (trig-moment) factorization.

Reference (per (b,t), B=16, T=64, N=512, grid 128x128, bandwidth h):
  standardize points (mean/std ddof=1 over N), then
  density[g,h] = 1/(2 pi h^2) * sum_n exp(-(xg-x_n)^2/2h^2) * exp(-(yg-y_n)^2/2h^2)
              = sum_n phi(xg - x_n) * phi(yg - y_n),   phi = 1D-normalized Gaussian.

Kernel idea: periodize phi with period L and truncate its Fourier series at K
harmonics.  With theta = 2*pi*x/L and the D = 2K+1 feature vector
v(x) = [1, cos(j*theta), sin(j*theta)]_{j=1..K}:
  phi(g - x) ~= sum_d U[d, g] * v_d(x)       (U solved host-side by lstsq)
  density    = Ux^T M Uy,   M[d, e] = sum_n v_d(x_n) v_e(y_n).
Per (b,t) the device work collapses to tiny matmuls:
  M2 = Vy^T Vx   (PE, contract n in 4 chunks of 128)
  Z  = M2^T-contract: lhsT=M2[e,d], rhs=U  -> Z[d, h]
  D  = lhsT=U[d, g] (const stationary), rhs=Z -> density[g, h]
V tiles [n, D] are built once per core for all 128 bt via half-angle Sin
seeds (ACT) + Chebyshev recurrences (DVE/Pool) in fp16 — no per-point exp.
Output is written f16 and upcast on host (halves output DMA traffic).

Validated in numpy: K=8, L=11 gives rel-Frobenius ~3e-3 (gate 2e-2).
"""

import math

import numpy as np

import concourse.bass as bass
import concourse.bacc as bacc
import concourse.mybir as mybir
from concourse import tile
from concourse.bass_utils import run_bass_kernel_spmd

B, T, N, GRID = 16, 64, 512, 128
NCORES = 8
BT_PER_CORE = (B * T) // NCORES  # 128
NCHUNK = N // 128  # 4

KHARM = 8            # Fourier harmonics
LPER = 11.0          # periodization length
DDIM = 2 * KHARM + 1  # 17 feature dims

F32 = mybir.dt.float32
F16 = mybir.dt.float16

_CACHE = {}


def _build(bw: float):
    AT = mybir.ActivationFunctionType
    OP = mybir.AluOpType

    nc = bacc.Bacc("TRN2", target_bir_lowering=False)
    a_ext = nc.declare_dram_parameter("a", [BT_PER_CORE, N, 2], F32, isOutput=False)
    idt_ext = nc.declare_dram_parameter("idt", [128, 128], F32, isOutput=False)
    u_ext = nc.declare_dram_parameter("umat", [DDIM, GRID], F16, isOutput=False)
    out_ext = nc.declare_dram_parameter(
        "out", [BT_PER_CORE, GRID, GRID], F16, isOutput=True
    )

    two_pi_over_l = 2.0 * math.pi / LPER

    with tile.TileContext(nc) as tc:
        with (
            tc.tile_pool(name="const", bufs=1) as cpool,
            tc.tile_pool(name="stats", bufs=1) as spool,
            tc.tile_pool(name="vpool", bufs=1) as vpool,
            tc.tile_pool(name="work", bufs=2) as wpool,
            tc.tile_pool(name="msb", bufs=2) as msbpool,
            tc.tile_pool(name="zsb", bufs=2) as zsbpool,
            tc.tile_pool(name="dsb", bufs=2) as dsbpool,
        ):
            idt_sb = cpool.tile([128, 128], F32, tag="idt")
            u_sb = cpool.tile([DDIM, GRID], F16, tag="umat")
            halfpi = cpool.tile([128, 1], F32, tag="halfpi")
            nc.sync.dma_start(idt_sb[:], idt_ext[:])
            nc.sync.dma_start(u_sb[:], u_ext[:])
            nc.vector.memset(halfpi[:], math.pi / 2.0)

            a_all = spool.tile([128, N, 2], F32, tag="a")
            nc.sync.dma_start(a_all[:], a_ext[:])

            # ---- per-bt stats; theta = (x - mean) * invsd * (2 pi / L) ----
            # layout [bt(128 part), n(512 free)]
            theta = {}
            for ch, ci in (("x", 0), ("y", 1)):
                src = a_all[:, :, ci]
                s1 = spool.tile([128, 1], F32, tag=f"s1{ch}")
                s2 = spool.tile([128, 1], F32, tag=f"s2{ch}")
                sq = wpool.tile([128, N], F32, tag=f"sq{ch}")
                nc.vector.tensor_reduce(s1[:], src, mybir.AxisListType.X, OP.add)
                # square-and-sum on ACT (accum_out) to keep DVE free
                nc.scalar.activation(sq[:], src, AT.Square, accum_out=s2[:])
                mean = spool.tile([128, 1], F32, tag=f"mean{ch}")
                nc.vector.tensor_scalar_mul(mean[:], s1[:], 1.0 / N)
                m2 = spool.tile([128, 1], F32, tag=f"m2{ch}")
                nc.vector.tensor_tensor(m2[:], mean[:], mean[:], OP.mult)
                var = spool.tile([128, 1], F32, tag=f"var{ch}")
                nc.vector.scalar_tensor_tensor(
                    var[:], m2[:], -float(N), s2[:], OP.mult, OP.add
                )
                nc.vector.tensor_scalar_mul(var[:], var[:], 1.0 / (N - 1))
                sd = spool.tile([128, 1], F32, tag=f"sd{ch}")
                nc.scalar.activation(sd[:], var[:], AT.Sqrt)
                invsd = spool.tile([128, 1], F32, tag=f"invsd{ch}")
                nc.vector.reciprocal(invsd[:], sd[:])
                alph = spool.tile([128, 1], F32, tag=f"alph{ch}")
                nc.vector.tensor_scalar_mul(alph[:], invsd[:], two_pi_over_l)
                th = spool.tile([128, N], F32, tag=f"th{ch}")
                # (x - mean) * alpha in one two-stage tensor_scalar
                nc.vector.tensor_scalar(
                    th[:], src, mean[:, 0:1], alph[:, 0:1], OP.subtract, OP.mult
                )
                theta[ch] = th

            # ---- transpose theta to [n(128 part), (chunk, bt)] ----
            thT = {}
            with tc.tile_pool(name="ptr", bufs=1, space="PSUM") as trpool:
                for ch in ("x", "y"):
                    pt = trpool.tile([128, NCHUNK, 128], F32, tag=f"pt{ch}")
                    for cc in range(NCHUNK):
                        nc.tensor.transpose(
                            pt[:, cc, :],
                            theta[ch][:, cc * 128 : (cc + 1) * 128],
                            idt_sb[:],
                        )
                    tt = vpool.tile([128, NCHUNK, 128], F32, tag=f"thT{ch}")
                    nc.scalar.activation(tt[:], pt[:], AT.Copy)
                    thT[ch] = tt

            # ---- V tiles [n part, d, chunk, bt] f16 via seeds + Chebyshev ----
            # v_0 = 1, v_j = cos(j th), v_{K+j} = sin(j th)
            V = {}
            for ch in ("x", "y"):
                v = vpool.tile([128, DDIM, NCHUNK, 128], F16, tag=f"V{ch}")
                nc.vector.memset(v[:, 0], 1.0)
                hs = wpool.tile([128, NCHUNK, 128], F16, tag=f"hs{ch}")
                hc = wpool.tile([128, NCHUNK, 128], F16, tag=f"hc{ch}")
                # half-angle seeds keep Sin args within [-pi, pi]
                nc.scalar.activation(hs[:], thT[ch][:], AT.Sin, scale=0.5)
                nc.scalar.activation(
                    hc[:], thT[ch][:], AT.Sin, bias=halfpi[:, 0:1], scale=0.5
                )
                # sin th = 2 hs hc ; cos th = 1 - 2 hs^2
                prod = wpool.tile([128, NCHUNK, 128], F16, tag=f"pr{ch}")
                nc.vector.tensor_tensor(prod[:], hs[:], hc[:], OP.mult)
                nc.vector.tensor_scalar(
                    v[:, KHARM + 1], prod[:], 2.0, None, OP.mult
                )
                nc.vector.tensor_tensor(prod[:], hs[:], hs[:], OP.mult)
                nc.vector.tensor_scalar(
                    v[:, 1], prod[:], -2.0, 1.0, OP.mult, OP.add
                )
                t2 = wpool.tile([128, NCHUNK, 128], F16, tag=f"t2{ch}")
                nc.vector.tensor_scalar(t2[:], v[:, 1], 2.0, None, OP.mult)
                V[ch] = (v, t2)

            vx, _ = V["x"]
            vy, _ = V["y"]

            # Chebyshev: v_{j} = t2 * v_{j-1} - v_{j-2} for both cos and sin
            # chains, emitted per bt-half so the first half of the bt loop can
            # overlap with the second half of the V build.
            def emit_recurrence(half):
                sl = slice(half * 64, half * 64 + 64)
                ops = []
                for j in range(2, KHARM + 1):
                    for ch in ("x", "y"):
                        v, t2 = V[ch]
                        for base in (0, KHARM):
                            jm1 = base + j - 1
                            jm2 = base + j - 2
                            dst = base + j
                            sin_start = base and j == 2  # sin_0 = 0
                            ops.append((ch, t2, v, jm1, jm2, dst, sin_start, base))
                insts = []
                for ch, t2, v, jm1, jm2, dst, sin_start, base in ops:
                    u = wpool.tile([128, NCHUNK, 64], F16, tag=f"u{ch}{dst}{half}")
                    insts.append(
                        ("tt", "dve", u[:], t2[:, :, sl], v[:, jm1, :, sl], OP.mult)
                    )
                    # sin-chain subtracts run on GPSIMD to offload DVE
                    sub_eng = "pool" if base else "dve"
                    if sin_start:
                        insts.append(("cp", "dve", v[:, dst, :, sl], u[:]))
                    else:
                        insts.append(
                            ("tt", sub_eng, v[:, dst, :, sl], u[:],
                             v[:, jm2, :, sl], OP.subtract)
                        )
                return insts

            def run_inst(inst):
                eng = nc.vector if inst[1] == "dve" else nc.gpsimd
                if inst[0] == "tt":
                    eng.tensor_tensor(inst[2], inst[3], inst[4], inst[5])
                else:
                    eng.tensor_copy(inst[2], inst[3])

            for inst in emit_recurrence(0):
                run_inst(inst)
            pending = emit_recurrence(1)

            # ---- main loop: groups of 4 bt; DMA out in 8-bt batches ----
            loop_pools = (
                tc.tile_pool(name="pm", bufs=2, space="PSUM"),
                tc.tile_pool(name="pz", bufs=3, space="PSUM"),
                tc.tile_pool(name="pd", bufs=3, space="PSUM"),
            )
            mpool = loop_pools[0].__enter__()
            zpool = loop_pools[1].__enter__()
            dpool = loop_pools[2].__enter__()
            NG = BT_PER_CORE // 4
            dsb = None
            for gi in range(NG):
                # interleave second-half V ops with the first half of the loop
                if gi < 16:
                    for _ in range(7):
                        if pending:
                            run_inst(pending.pop(0))
                mps = mpool.tile([DDIM, 4, DDIM], F32, tag="mps")
                for i in range(4):
                    bt = gi * 4 + i
                    for cc in range(NCHUNK):
                        # M2[e,d] = sum_n Vy[n,e] Vx[n,d]
                        nc.tensor.matmul(
                            mps[:, i, :],
                            vy[:, :, cc, bt],
                            vx[:, :, cc, bt],
                            start=(cc == 0),
                            stop=(cc == NCHUNK - 1),
                        )
                msb = msbpool.tile([DDIM, 4, DDIM], F16, tag="msb")
                nc.vector.tensor_copy(msb[:], mps[:])

                zps = zpool.tile([DDIM, 4, GRID], F32, tag="zps")
                for i in range(4):
                    # Z[d, h] = sum_e M2[e, d] U[e, h]
                    nc.tensor.matmul(
                        zps[:, i, :], msb[:, i, :], u_sb[:], start=True, stop=True
                    )
                zsb = zsbpool.tile([DDIM, 4, GRID], F16, tag="zsb")
                nc.scalar.activation(zsb[:], zps[:], AT.Copy)

                dps = dpool.tile([128, 4, GRID], F32, tag="dps")
                # density[g, h] = sum_d U[d, g] Z[d, h] — one matmul per group
                nc.tensor.matmul(dps[:], u_sb[:], zsb[:], start=True, stop=True)
                if gi % 2 == 0:
                    dsb = dsbpool.tile([128, 8, GRID], F16, tag="dsb")
                half = dsb[:, (gi % 2) * 4 : (gi % 2) * 4 + 4, :]
                # D-evac: ~2/3 on DVE, 1/3 on ACT (ACT carries all Z-evacs)
                if gi % 3 == 2:
                    nc.scalar.activation(half, dps[:], AT.Copy)
                else:
                    nc.vector.tensor_copy(half, dps[:])
                if gi % 2 == 1:
                    dst = out_ext[(gi - 1) * 4 : (gi + 1) * 4].transpose([1, 0, 2])
                    nc.sync.dma_start(dst, dsb[:])
            while pending:
                run_inst(pending.pop(0))
            for cm in reversed(loop_pools):
                cm.__exit__(None, None, None)

    if not nc.is_finalized():
        nc.finalize()
    return nc


def _consts(bw: float):
    h = float(bw)
    g = np.linspace(-5.0, 5.0, GRID)
    xs = np.linspace(-5.1, 5.1, 4001)
    th = 2.0 * np.pi * xs / LPER
    cols = (
        [np.ones_like(th)]
        + [np.cos(j * th) for j in range(1, KHARM + 1)]
        + [np.sin(j * th) for j in range(1, KHARM + 1)]
    )
    Phi = np.stack(cols, axis=-1)  # [S, D]
    Tgt = np.exp(-((g[None, :] - xs[:, None]) ** 2) / (2.0 * h * h)) / (
        np.sqrt(2.0 * np.pi) * h
    )
    AtA = Phi.T @ Phi + 1e-7 * len(xs) * np.eye(DDIM)
    U = np.linalg.solve(AtA, Phi.T @ Tgt)  # [D, G]
    idt = np.eye(128, dtype=np.float32)
    return idt, U.astype(np.float16)


def kernel(A: np.ndarray, bandwidth: np.ndarray) -> np.ndarray:
    A = np.asarray(A, dtype=np.float32)
    bw = float(np.asarray(bandwidth))
    key = round(bw, 9)
    if key not in _CACHE:
        _CACHE[key] = _build(bw)
    nc = _CACHE[key]

    idt, umat = _consts(bw)
    a_flat = A.reshape(B * T, N, 2)
    in_maps = []
    for i in range(NCORES):
        in_maps.append(
            {
                "a": np.ascontiguousarray(
                    a_flat[i * BT_PER_CORE : (i + 1) * BT_PER_CORE]
                ),
                "idt": idt,
                "umat": umat,
            }
        )
    res = run_bass_kernel_spmd(nc, in_maps, core_ids=list(range(NCORES)))
    outs = [res.results[i]["out"] for i in range(NCORES)]
    return (
        np.concatenate(outs, axis=0).astype(np.float32).reshape(B, T, GRID, GRID)
    )


if __name__ == "__main__":
    Arand = np.random.randn(B, T, N, 2).astype(np.float32)
    out = kernel(Arand, np.float32(0.5))
    print(out.shape, out.dtype, float(out.max()))


# revision 20
# speedup vs baseline: 1.1087x; 1.0520x over previous
"""KDE2D Trainium2 Bass kernel — Fourier (trig-moment) factorization.

Reference (per (b,t), B=16, T=64, N=512, grid 128x128, bandwidth h):
  standardize points (mean/std ddof=1 over N), then
  density[g,h] = 1/(2 pi h^2) * sum_n exp(-(xg-x_n)^2/2h^2) * exp(-(yg-y_n)^2/2h^2)
              = sum_n phi(xg - x_n) * phi(yg - y_n),   phi = 1D-normalized Gaussian.

Kernel idea: periodize phi with period L and truncate its Fourier series at K
harmonics.  With theta = 2*pi*x/L and the D = 2K+1 feature vector
v(x) = [1, cos(j*theta), sin(j*theta)]_{j=1..K}:
  phi(g - x) ~= sum_d U[d, g] * v_d(x)       (U solved host-side by lstsq)
  density    = Ux^T M Uy,   M[d, e] = sum_n v_d(x_n) v_e(y_n).
Per (b,t) the device work collapses to tiny matmuls:
  M2 = Vy^T Vx   (PE, contract n in 4 chunks of 128)
  Z  = M2^T-contract: lhsT=M2[e,d], rhs=U  -> Z[d, h]
  D  = lhsT=U[d, g] (const stationary), rhs=Z -> density[g, h]
V tiles [n, D] are built once per core for all 128 bt via half-angle Sin
seeds (ACT) + Chebyshev recurrences (DVE/Pool) in fp16 — no per-point exp.
Output is written f16 and upcast on host (halves output DMA traffic).

Validated in numpy: K=8, L=11 gives rel-Frobenius ~3e-3 (gate 2e-2).
"""

import math

import numpy as np

import concourse.bass as bass
import concourse.bacc as bacc
import concourse.mybir as mybir
from concourse import tile
from concourse.bass_utils import run_bass_kernel_spmd

B, T, N, GRID = 16, 64, 512, 128
NCORES = 8
BT_PER_CORE = (B * T) // NCORES  # 128
NCHUNK = N // 128  # 4

KHARM = 8            # Fourier harmonics
LPER = 11.0          # periodization length
DDIM = 2 * KHARM + 1  # 17 feature dims

F32 = mybir.dt.float32
F16 = mybir.dt.float16

_CACHE = {}


def _build(bw: float):
    AT = mybir.ActivationFunctionType
    OP = mybir.AluOpType

    nc = bacc.Bacc("TRN2", target_bir_lowering=False)
    a_ext = nc.declare_dram_parameter("a", [BT_PER_CORE, N, 2], F32, isOutput=False)
    idt_ext = nc.declare_dram_parameter("idt", [128, 128], F32, isOutput=False)
    u_ext = nc.declare_dram_parameter("umat", [DDIM, GRID], F16, isOutput=False)
    out_ext = nc.declare_dram_parameter(
        "out", [BT_PER_CORE, GRID, GRID], F16, isOutput=True
    )

    two_pi_over_l = 2.0 * math.pi / LPER

    with tile.TileContext(nc) as tc:
        with (
            tc.tile_pool(name="const", bufs=1) as cpool,
            tc.tile_pool(name="stats", bufs=1) as spool,
            tc.tile_pool(name="vpool", bufs=1) as vpool,
            tc.tile_pool(name="work", bufs=2) as wpool,
            tc.tile_pool(name="msb", bufs=2) as msbpool,
            tc.tile_pool(name="zsb", bufs=2) as zsbpool,
            tc.tile_pool(name="dsb", bufs=2) as dsbpool,
        ):
            idt_sb = cpool.tile([128, 128], F32, tag="idt")
            u_sb = cpool.tile([DDIM, GRID], F16, tag="umat")
            halfpi = cpool.tile([128, 1], F32, tag="halfpi")
            nc.sync.dma_start(idt_sb[:], idt_ext[:])
            nc.sync.dma_start(u_sb[:], u_ext[:])
            nc.vector.memset(halfpi[:], math.pi / 2.0)

            a_all = spool.tile([128, N, 2], F32, tag="a")
            nc.sync.dma_start(a_all[:], a_ext[:])

            # ---- per-bt stats; theta = (x - mean) * invsd * (2 pi / L) ----
            # layout [bt(128 part), n(512 free)]
            theta = {}
            for ch, ci in (("x", 0), ("y", 1)):
                src = a_all[:, :, ci]
                s1 = spool.tile([128, 1], F32, tag=f"s1{ch}")
                s2 = spool.tile([128, 1], F32, tag=f"s2{ch}")
                sq = wpool.tile([128, N], F32, tag=f"sq{ch}")
                nc.vector.tensor_reduce(s1[:], src, mybir.AxisListType.X, OP.add)
                # square-and-sum on ACT (accum_out) to keep DVE free
                nc.scalar.activation(sq[:], src, AT.Square, accum_out=s2[:])
                mean = spool.tile([128, 1], F32, tag=f"mean{ch}")
                nc.vector.tensor_scalar_mul(mean[:], s1[:], 1.0 / N)
                m2 = spool.tile([128, 1], F32, tag=f"m2{ch}")
                nc.vector.tensor_tensor(m2[:], mean[:], mean[:], OP.mult)
                var = spool.tile([128, 1], F32, tag=f"var{ch}")
                nc.vector.scalar_tensor_tensor(
                    var[:], m2[:], -float(N), s2[:], OP.mult, OP.add
                )
                nc.vector.tensor_scalar_mul(var[:], var[:], 1.0 / (N - 1))
                sd = spool.tile([128, 1], F32, tag=f"sd{ch}")
                nc.scalar.activation(sd[:], var[:], AT.Sqrt)
                invsd = spool.tile([128, 1], F32, tag=f"invsd{ch}")
                nc.vector.reciprocal(invsd[:], sd[:])
                alph = spool.tile([128, 1], F32, tag=f"alph{ch}")
                nc.vector.tensor_scalar_mul(alph[:], invsd[:], two_pi_over_l)
                th = spool.tile([128, N], F32, tag=f"th{ch}")
                # (x - mean) * alpha in one two-stage tensor_scalar
                nc.vector.tensor_scalar(
                    th[:], src, mean[:, 0:1], alph[:, 0:1], OP.subtract, OP.mult
                )
                theta[ch] = th

            # ---- transpose theta to [n(128 part), (chunk, bt)] ----
            thT = {}
            with tc.tile_pool(name="ptr", bufs=1, space="PSUM") as trpool:
                for ch in ("x", "y"):
                    pt = trpool.tile([128, NCHUNK, 128], F32, tag=f"pt{ch}")
                    for cc in range(NCHUNK):
                        nc.tensor.transpose(
                            pt[:, cc, :],
                            theta[ch][:, cc * 128 : (cc + 1) * 128],
                            idt_sb[:],
                        )
                    tt = vpool.tile([128, NCHUNK, 128], F32, tag=f"thT{ch}")
                    nc.scalar.activation(tt[:], pt[:], AT.Copy)
                    thT[ch] = tt

            # ---- V tiles [n part, d, chunk, bt] f16 via seeds + Chebyshev ----
            # v_0 = 1, v_j = cos(j th), v_{K+j} = sin(j th)
            V = {}
            for ch in ("x", "y"):
                v = vpool.tile([128, DDIM, NCHUNK, 128], F16, tag=f"V{ch}")
                nc.vector.memset(v[:, 0], 1.0)
                hs = wpool.tile([128, NCHUNK, 128], F16, tag=f"hs{ch}")
                hc = wpool.tile([128, NCHUNK, 128], F16, tag=f"hc{ch}")
                # half-angle seeds keep Sin args within [-pi, pi]
                nc.scalar.activation(hs[:], thT[ch][:], AT.Sin, scale=0.5)
                nc.scalar.activation(
                    hc[:], thT[ch][:], AT.Sin, bias=halfpi[:, 0:1], scale=0.5
                )
                # sin th = 2 hs hc ; cos th = 1 - 2 hs^2
                prod = wpool.tile([128, NCHUNK, 128], F16, tag=f"pr{ch}")
                nc.vector.tensor_tensor(prod[:], hs[:], hc[:], OP.mult)
                nc.vector.tensor_scalar(
                    v[:, KHARM + 1], prod[:], 2.0, None, OP.mult
                )
                nc.vector.tensor_tensor(prod[:], hs[:], hs[:], OP.mult)
                nc.vector.tensor_scalar(
                    v[:, 1], prod[:], -2.0, 1.0, OP.mult, OP.add
                )
                t2 = wpool.tile([128, NCHUNK, 128], F16, tag=f"t2{ch}")
                nc.vector.tensor_scalar(t2[:], v[:, 1], 2.0, None, OP.mult)
                V[ch] = (v, t2)

            vx, _ = V["x"]
            vy, _ = V["y"]

            # Chebyshev: v_{j} = t2 * v_{j-1} - v_{j-2} for both cos and sin
            # chains, emitted per bt-half so the first half of the bt loop can
            # overlap with the second half of the V build.
            def emit_recurrence(half):
                sl = slice(half * 64, half * 64 + 64)
                ops = []
                for j in range(2, KHARM + 1):
                    for ch in ("x", "y"):
                        v, t2 = V[ch]
                        for base in (0, KHARM):
                            jm1 = base + j - 1
                            jm2 = base + j - 2
                            dst = base + j
                            sin_start = base and j == 2  # sin_0 = 0
                            ops.append((ch, t2, v, jm1, jm2, dst, sin_start, base))
                insts = []
                for ch, t2, v, jm1, jm2, dst, sin_start, base in ops:
                    u = wpool.tile([128, NCHUNK, 64], F16, tag=f"u{ch}{dst}{half}")
                    insts.append(
                        ("tt", "dve", u[:], t2[:, :, sl], v[:, jm1, :, sl], OP.mult)
                    )
                    # sin-chain subtracts run on GPSIMD to offload DVE
                    sub_eng = "pool" if base else "dve"
                    if sin_start:
                        insts.append(("cp", "dve", v[:, dst, :, sl], u[:]))
                    else:
                        insts.append(
                            ("tt", sub_eng, v[:, dst, :, sl], u[:],
                             v[:, jm2, :, sl], OP.subtract)
                        )
                return insts

            def run_inst(inst):
                eng = nc.vector if inst[1] == "dve" else nc.gpsimd
                if inst[0] == "tt":
                    eng.tensor_tensor(inst[2], inst[3], inst[4], inst[5])
                else:
                    eng.tensor_copy(inst[2], inst[3])

            for inst in emit_recurrence(0):
                run_inst(inst)
            pending = emit_recurrence(1)

            # ---- main loop: groups of 4 bt; DMA out in 8-bt batches ----
            loop_pools = (
                tc.tile_pool(name="pm", bufs=2, space="PSUM"),
                tc.tile_pool(name="pz", bufs=3, space="PSUM"),
                tc.tile_pool(name="pd", bufs=3, space="PSUM"),
            )
            mpool = loop_pools[0].__enter__()
            zpool = loop_pools[1].__enter__()
            dpool = loop_pools[2].__enter__()
            NG = BT_PER_CORE // 4
            dsb = None
            for gi in range(NG):
                # interleave second-half V ops with the first half of the loop
                for _ in range(8):
                    if pending:
                        run_inst(pending.pop(0))
                mps = mpool.tile([DDIM, 4, DDIM], F32, tag="mps")
                for i in range(4):
                    bt = gi * 4 + i
                    for cc in range(NCHUNK):
                        # M2[e,d] = sum_n Vy[n,e] Vx[n,d]
                        nc.tensor.matmul(
                            mps[:, i, :],
                            vy[:, :, cc, bt],
                            vx[:, :, cc, bt],
                            start=(cc == 0),
                            stop=(cc == NCHUNK - 1),
                        )
                msb = msbpool.tile([DDIM, 4, DDIM], F16, tag="msb")
                # while the V build still occupies DVE, evacuate M on ACT
                if gi < 16:
                    nc.scalar.activation(msb[:], mps[:], AT.Copy)
                else:
                    nc.vector.tensor_copy(msb[:], mps[:])

                zps = zpool.tile([DDIM, 4, GRID], F32, tag="zps")
                for i in range(4):
                    # Z[d, h] = sum_e M2[e, d] U[e, h]
                    nc.tensor.matmul(
                        zps[:, i, :], msb[:, i, :], u_sb[:], start=True, stop=True
                    )
                zsb = zsbpool.tile([DDIM, 4, GRID], F16, tag="zsb")
                nc.scalar.activation(zsb[:], zps[:], AT.Copy)

                dps = dpool.tile([128, 4, GRID], F32, tag="dps")
                # density[g, h] = sum_d U[d, g] Z[d, h] — one matmul per group
                nc.tensor.matmul(dps[:], u_sb[:], zsb[:], start=True, stop=True)
                if gi % 2 == 0:
                    dsb = dsbpool.tile([128, 8, GRID], F16, tag="dsb")
                half = dsb[:, (gi % 2) * 4 : (gi % 2) * 4 + 4, :]
                # D-evac: half/half while V build loads DVE, then 2/3 DVE
                on_act = (gi % 2 == 1) if gi < 16 else (gi % 3 == 2)
                if on_act:
                    nc.scalar.activation(half, dps[:], AT.Copy)
                else:
                    nc.vector.tensor_copy(half, dps[:])
                if gi % 2 == 1:
                    dst = out_ext[(gi - 1) * 4 : (gi + 1) * 4].transpose([1, 0, 2])
                    nc.sync.dma_start(dst, dsb[:])
            while pending:
                run_inst(pending.pop(0))
            for cm in reversed(loop_pools):
                cm.__exit__(None, None, None)

    if not nc.is_finalized():
        nc.finalize()
    return nc


def _consts(bw: float):
    h = float(bw)
    g = np.linspace(-5.0, 5.0, GRID)
    xs = np.linspace(-5.1, 5.1, 4001)
    th = 2.0 * np.pi * xs / LPER
    cols = (
        [np.ones_like(th)]
        + [np.cos(j * th) for j in range(1, KHARM + 1)]
        + [np.sin(j * th) for j in range(1, KHARM + 1)]
    )
    Phi = np.stack(cols, axis=-1)  # [S, D]
    Tgt = np.exp(-((g[None, :] - xs[:, None]) ** 2) / (2.0 * h * h)) / (
        np.sqrt(2.0 * np.pi) * h
    )
    AtA = Phi.T @ Phi + 1e-7 * len(xs) * np.eye(DDIM)
    U = np.linalg.solve(AtA, Phi.T @ Tgt)  # [D, G]
    idt = np.eye(128, dtype=np.float32)
    return idt, U.astype(np.float16)


def kernel(A: np.ndarray, bandwidth: np.ndarray) -> np.ndarray:
    A = np.asarray(A, dtype=np.float32)
    bw = float(np.asarray(bandwidth))
    key = round(bw, 9)
    if key not in _CACHE:
        _CACHE[key] = _build(bw)
    nc = _CACHE[key]

    idt, umat = _consts(bw)
    a_flat = A.reshape(B * T, N, 2)
    in_maps = []
    for i in range(NCORES):
        in_maps.append(
            {
                "a": np.ascontiguousarray(
                    a_flat[i * BT_PER_CORE : (i + 1) * BT_PER_CORE]
                ),
                "idt": idt,
                "umat": umat,
            }
        )
    res = run_bass_kernel_spmd(nc, in_maps, core_ids=list(range(NCORES)))
    outs = [res.results[i]["out"] for i in range(NCORES)]
    return (
        np.concatenate(outs, axis=0).astype(np.float32).reshape(B, T, GRID, GRID)
    )


if __name__ == "__main__":
    Arand = np.random.randn(B, T, N, 2).astype(np.float32)
    out = kernel(Arand, np.float32(0.5))
    print(out.shape, out.dtype, float(out.max()))
